# revision 20
# baseline (speedup 1.0000x reference)
"""Trainium2 Bass kernel for nn_ExplicitGCN_90829968375999 (v2, feature-major).

Math (same as v1, verified vs reference):
  The reference tiles edge_index B times with UNCHANGED node ids in [0,V),
  so all E*B edge messages act only on the batch-0 block of the flattened
  B*V node array. Batches 1..3 see only their self-loops (deg=1) and
  degenerate to a per-node MLP. For batch 0, with deg = B*indeg+1 and
  dis = deg**-0.5, the layer update is
      u[v]  = B*dis[v] * sum_{e: dst=v} g[src_e]  +  dis[v]^2 * h[v]
      x'[v] = relu(u[v] + b_l),        g = dis * h,   h = x @ W_l.T

Distribution (v2): 8 cores, each owns V/8=6250 nodes. Activations stay
FEATURE-major throughout. Per layer: G = dis*(W_l @ X) computed feature-
major, one XBAR dma-transpose builds the node-major table slice, a
shared-output AllGather assembles the 50176-row bf16 table in DRAM, and
transpose-mode dma_gather launches (4 SWDGE queues) pull neighbor rows
back FEATURE-major so the segment-sums (contiguous innermost-k Vector
reduces), the self term and the relu all happen without any per-tile
PE-array transposes. Per-column dis scaling uses host-precomputed
[128, PER_CORE] broadcast arrays; the B=4 factor is folded into the
activation's imm scale and dis^2/4 into the self-term array.
"""

import os
import sys
import types
import numpy as np

NCORES = 8
V, E, B, LAT, H = 50000, 300000, 4, 512, 128
NT = 49                        # 128-node tiles per core
PER_CORE = NT * 128            # 6272 owned rows (incl. dummies)
REAL_PER_CORE = V // NCORES    # 6250
TBL_ROWS = NCORES * PER_CORE   # 50176
W2_BASE = TBL_ROWS - 32768     # 17408
LO_FIX = 3                     # src cores 0..2 -> window 1 only
HI_FIX = 5                     # src cores 5..7 -> window 2 only; 3,4 flexible
# table row index of node (core c, position i=(t*128+j)) is c*6272 + j*49 + t
PAD_LO = 106 * NT + 48                         # core-0 dummy row (zero)
PAD_HI = 7 * PER_CORE + 127 * NT + 48 - W2_BASE  # core-7 dummy row, w2-rel
NCHUNK = 512                   # matmul moving-dim chunk
GROUP_BLOCKS = int(os.environ.get("GCN_GROUP_BLOCKS", "32"))
NQUEUES = 4

RED_DT = os.environ.get("GCN_RED_DT", "bf16")   # reduce output dtype

_CACHE = {}
LAST_EXEC_NS = None


def _install_ntff_hook():
    """Best-effort shim for the missing antenv.axon_hooks module so
    run_bass_kernel_spmd(trace=True) can capture an NTFF profile."""
    try:
        import antenv  # noqa: F401
        try:
            from antenv.axon_hooks import get_axon_ntff_profile_hook  # noqa: F401
            return
        except ImportError:
            pass
        mod = types.ModuleType("antenv.axon_hooks")
        mod._HOOK = None

        def set_axon_ntff_profile_hook(h):
            mod._HOOK = h

        def get_axon_ntff_profile_hook():
            return mod._HOOK

        mod.set_axon_ntff_profile_hook = set_axon_ntff_profile_hook
        mod.get_axon_ntff_profile_hook = get_axon_ntff_profile_hook
        sys.modules["antenv.axon_hooks"] = mod
        import antenv as _a
        _a.axon_hooks = mod
        from trn_agent_boot.trn_boot import _ntff_profile_via_ctypes
        set_axon_ntff_profile_hook(
            _ntff_profile_via_ctypes("/opt/axon/libaxon_pjrt.so")
        )
    except Exception:
        pass


def _preprocess(edge_index):
    """Graph structure -> per-core node order, per-tile uniform block counts
    and j-major int16 gather index arrays. Pure int bookkeeping."""
    src = np.asarray(edge_index[0], dtype=np.int64)
    dst = np.asarray(edge_index[1], dtype=np.int64)

    indeg = np.bincount(dst, minlength=V)

    # core assignment: snake-deal nodes in descending-degree order
    by_deg = np.argsort(-indeg, kind="stable")
    seq = np.arange(V)
    row, col = seq // NCORES, seq % NCORES
    core_seq = np.where(row % 2 == 0, col, NCORES - 1 - col)
    core_of = np.empty(V, np.int64)
    core_of[by_deg] = core_seq

    # classify edges by source window: fixed-lo / flexible / fixed-hi
    sc = core_of[src]
    lo_fix_cnt = np.bincount(dst[sc < LO_FIX], minlength=V)
    hi_fix_cnt = np.bincount(dst[sc >= HI_FIX], minlength=V)
    flex_cnt = indeg - lo_fix_cnt - hi_fix_cnt

    # split flexible edges to balance lo/hi per node
    x = np.clip((hi_fix_cnt + flex_cnt - lo_fix_cnt + 1) // 2, 0, flex_cnt)
    lo_cnt = lo_fix_cnt + x
    hi_cnt = indeg - lo_cnt

    # per-edge lo/hi assignment: flex edges ranked within node, first x -> lo
    order_d = np.argsort(dst, kind="stable")
    src_d = src[order_d]
    dst_d = dst[order_d]
    sc_d = sc[order_d]
    is_flex = (sc_d >= LO_FIX) & (sc_d < HI_FIX)
    rowptr = np.zeros(V + 1, np.int64)
    rowptr[1:] = np.cumsum(indeg)
    flex_cum = np.cumsum(is_flex)
    flex_before_node = np.concatenate([[0], flex_cum])[rowptr[dst_d]]
    flex_rank = np.where(is_flex, flex_cum - 1 - flex_before_node, 0)
    edge_is_lo = (sc_d < LO_FIX) | (is_flex & (flex_rank < x[dst_d]))

    # re-sort edges by (dst, hi-ness) so lo edges come first per node
    order2 = np.argsort(dst_d * 2 + (~edge_is_lo).astype(np.int64), kind="stable")
    src_sorted = src_d[order2]

    # per-core node order: sort by (max(lo,hi), hi) for min tile padding
    mx = np.maximum(lo_cnt, hi_cnt)
    nodes = np.full((NCORES, PER_CORE), -1, np.int64)
    for c in range(NCORES):
        mine = np.where(core_of == c)[0]
        assert len(mine) == REAL_PER_CORE
        oc = mine[np.lexsort((hi_cnt[mine], mx[mine]))]
        nodes[c, :REAL_PER_CORE] = oc

    # table row index of each node: c*6272 + j*49 + t for position i=(t,j)
    tbl_idx = np.full(V, -1, np.int64)
    for c in range(NCORES):
        rn = nodes[c, :REAL_PER_CORE]
        i = np.arange(REAL_PER_CORE)
        tbl_idx[rn] = c * PER_CORE + (i % 128) * NT + (i // 128)

    # uniform per-tile block counts (max over cores so one SPMD program fits)
    lc = np.where(nodes >= 0, lo_cnt[np.maximum(nodes, 0)], 0).reshape(NCORES, NT, 128)
    hc = np.where(nodes >= 0, hi_cnt[np.maximum(nodes, 0)], 0).reshape(NCORES, NT, 128)
    Klo = lc.max(axis=(0, 2))
    Khi = hc.max(axis=(0, 2))
    assert (Klo + Khi).max() <= GROUP_BLOCKS, (Klo + Khi).max()

    # groups of tiles, each <= GROUP_BLOCKS blocks in the slot buffer
    groups = []
    t0 = 0
    while t0 < NT:
        t1, blo, bhi = t0, 0, 0
        while t1 < NT:
            nl, nh = blo + Klo[t1], bhi + Khi[t1]
            if nl + (nl % 2) + nh > GROUP_BLOCKS:
                break
            blo, bhi = nl, nh
            t1 += 1
        groups.append((t0, t1))
        t0 = t1

    n_lo_blocks = int(Klo.sum())
    n_hi_blocks = int(Khi.sum())

    # gather index arrays per core: per tile j-major [j (128), k (K[t])]
    lo_idx = np.full((NCORES, n_lo_blocks * 128), PAD_LO, np.int64)
    hi_idx = np.full((NCORES, n_hi_blocks * 128), PAD_HI, np.int64)
    lo_off = np.concatenate([[0], np.cumsum(Klo)])
    hi_off = np.concatenate([[0], np.cumsum(Khi)])
    for c in range(NCORES):
        nc_nodes = nodes[c].reshape(NT, 128)
        for t in range(NT):
            tn = nc_nodes[t]
            valid = tn >= 0
            tnv = np.maximum(tn, 0)
            l_c = np.where(valid, lo_cnt[tnv], 0)
            h_c = np.where(valid, hi_cnt[tnv], 0)
            base = rowptr[tnv]
            kl, kh = int(Klo[t]), int(Khi[t])
            if kl:
                blk = np.full((128, kl), PAD_LO, np.int64)
                for k in range(kl):
                    m = k < l_c
                    if m.any():
                        blk[m, k] = tbl_idx[src_sorted[base[m] + k]]
                lo_idx[c, lo_off[t] * 128:(lo_off[t] + kl) * 128] = blk.T.reshape(-1)
            if kh:
                blk = np.full((128, kh), PAD_HI + W2_BASE, np.int64)
                for k in range(kh):
                    m = k < h_c
                    if m.any():
                        blk[m, k] = tbl_idx[src_sorted[base[m] + l_c[m] + k]]
                hi_idx[c, hi_off[t] * 128:(hi_off[t] + kh) * 128] = \
                    blk.T.reshape(-1) - W2_BASE
    assert lo_idx.min() >= 0 and lo_idx.max() < 32768
    assert hi_idx.min() >= 0 and hi_idx.max() < 32768

    def wrap(a):  # flat (n,) -> (128, n/16) int16 wrapped, replicated x8
        w16 = a.reshape(-1, 16).T.astype(np.int16)
        return np.tile(w16, (8, 1))

    lo_wrapped = np.stack([wrap(lo_idx[c]) for c in range(NCORES)])
    hi_wrapped = np.stack([wrap(hi_idx[c]) for c in range(NCORES)])

    # per-core dis rows (0 for dummies -> zero table rows, zero self term)
    deg = np.where(nodes >= 0, B * indeg[np.maximum(nodes, 0)] + 1, 1).astype(np.float64)
    dis = 1.0 / np.sqrt(deg)
    dis[nodes < 0] = 0.0
    dis_row = dis.astype(np.float32)            # (NCORES, PER_CORE)

    return dict(
        nodes=nodes, Klo=Klo, Khi=Khi, groups=groups,
        n_lo_blocks=n_lo_blocks, n_hi_blocks=n_hi_blocks,
        lo_off=lo_off, hi_off=hi_off,
        lo_wrapped=lo_wrapped, hi_wrapped=hi_wrapped,
        dis_row=dis_row,
    )


def _build(meta):
    import concourse.bacc as bacc
    import concourse.mybir as mybir
    import concourse.tile as tile
    from concourse.bass import _add_dep_helper

    f32 = mybir.dt.float32
    f32r = mybir.dt.float32r
    bf16 = mybir.dt.bfloat16
    mdt = f32r
    rdt = bf16 if RED_DT == "bf16" else f32
    AF = mybir.ActivationFunctionType
    OP = mybir.AluOpType
    AX = mybir.AxisListType

    Klo, Khi = meta["Klo"], meta["Khi"]
    groups = meta["groups"]
    n_lo_blocks, n_hi_blocks = meta["n_lo_blocks"], meta["n_hi_blocks"]
    lo_off, hi_off = meta["lo_off"], meta["hi_off"]

    nc = bacc.Bacc("TRN2", target_bir_lowering=False, debug=False,
                   num_devices=NCORES, num_swdge_queues=NQUEUES)

    # ---- external inputs ------------------------------------------------
    p_xyzT = nc.declare_dram_parameter("xyzT", [3, PER_CORE], mdt, isOutput=False)
    p_disb = nc.declare_dram_parameter("disb", [128, PER_CORE], f32, isOutput=False)
    p_dispp = nc.declare_dram_parameter("dis_pp", [128, NT], f32, isOutput=False)
    p_disBpp = nc.declare_dram_parameter("disB_pp", [128, NT], f32, isOutput=False)
    p_loidx = nc.declare_dram_parameter("lo_idx", [128, n_lo_blocks * 8],
                                        mybir.dt.int16, isOutput=False)
    p_hiidx = nc.declare_dram_parameter("hi_idx", [128, n_hi_blocks * 8],
                                        mybir.dt.int16, isOutput=False)
    p_wxyzT = nc.declare_dram_parameter("wxyzT", [3, 128], f32, isOutput=False)
    p_wlatT = nc.declare_dram_parameter("wlatT", [LAT, 128], f32, isOutput=False)
    p_latT = nc.declare_dram_parameter("latT", [LAT, B], f32, isOutput=False)
    p_bin = nc.declare_dram_parameter("b_in", [128, 1], f32, isOutput=False)
    p_convWT = nc.declare_dram_parameter("convWT", [3 * 128, 128], f32, isOutput=False)
    p_convbT = nc.declare_dram_parameter("convbT", [128, 3], f32, isOutput=False)
    p_woutT = nc.declare_dram_parameter("woutT", [128, 3], f32, isOutput=False)
    p_bout = nc.declare_dram_parameter("b_out", [3, 1], f32, isOutput=False)
    p_eye = nc.declare_dram_parameter("eye", [128, 128], f32, isOutput=False)
    p_out = nc.declare_dram_parameter("out_all", [B, 3, PER_CORE], f32, isOutput=True)
    DEBUG = os.environ.get("GCN_DEBUG", "0") == "1"
    if DEBUG:
        p_dbg_x0 = nc.declare_dram_parameter("dbg_x0", [128, PER_CORE], mdt, isOutput=True)
        p_dbg_g = nc.declare_dram_parameter("dbg_g", [128, PER_CORE], bf16, isOutput=True)
        p_dbg_tbl = nc.declare_dram_parameter("dbg_tbl", [TBL_ROWS, 128], bf16, isOutput=True)
        p_dbg_s = nc.declare_dram_parameter("dbg_s", [128, GROUP_BLOCKS * 128], bf16, isOutput=True)
        p_dbg_x1 = nc.declare_dram_parameter("dbg_x1", [128, PER_CORE], mdt, isOutput=True)
        p_dbg_tn = nc.declare_dram_parameter("dbg_tn", [128, NT * 128], bf16, isOutput=True)
        p_dbg_go = nc.declare_dram_parameter("dbg_go", [PER_CORE, 128], bf16, isOutput=True)

    with tile.TileContext(nc) as tc:
        with (
            tc.tile_pool(name="persist", bufs=1) as pp,
            tc.tile_pool(name="work", bufs=3) as wk,
            tc.tile_pool(name="slots", bufs=int(os.environ.get("GCN_SLOT_BUFS", "6"))) as sl,
            tc.tile_pool(name="dense", bufs=2) as dn,
            tc.tile_pool(name="ps_mm", bufs=3, space="PSUM") as ps_mm,
            tc.tile_pool(name="ps_t", bufs=2, space="PSUM") as ps_t,
            tc.tile_pool(name="ps_c", bufs=1, space="PSUM") as ps_c,
            tc.tile_pool(name="dram", bufs=1, space="DRAM") as dr,
        ):
            # ---- persistent SBUF state ---------------------------------
            # All persistent slots are padded to 512B multiples so SWDGE
            # gather / XBAR transpose operands (and later pool bases) stay
            # 512B-aligned — SBUF offset misalignment corrupts those paths.
            rup = lambda a, m: (a + m - 1) // m * m
            tblnm = pp.tile([128, NT, 128], bf16,
                            padded_shape=[128, NT + 1, 128])  # XBAR out
            wlatT = pp.tile([128, 4, 128], f32)
            nc.sync.dma_start(wlatT[:], p_wlatT[:].rearrange("(c k) m -> k c m", k=128))
            latT = pp.tile([128, 4, B], f32, padded_shape=[128, 4, 32])
            nc.sync.dma_start(latT[:], p_latT[:].rearrange("(c k) b -> k c b", k=128))
            b_in = pp.tile([128, 1], f32, padded_shape=[128, 128])
            nc.sync.dma_start(b_in[:], p_bin[:])
            convbT = pp.tile([128, 3], f32, padded_shape=[128, 128])
            nc.sync.dma_start(convbT[:], p_convbT[:])
            b_out = pp.tile([3, 1], f32, padded_shape=[3, 128])
            nc.sync.dma_start(b_out[:], p_bout[:])
            disb = pp.tile([128, PER_CORE], f32)
            nc.sync.dma_start(disb[:], p_disb[:])
            dis_pp = pp.tile([128, NT], f32, padded_shape=[128, 128])
            nc.sync.dma_start(dis_pp[:], p_dispp[:])
            disB_pp = pp.tile([128, NT], f32, padded_shape=[128, 128])
            nc.sync.dma_start(disB_pp[:], p_disBpp[:])
            lo_idx = pp.tile([128, n_lo_blocks * 8], mybir.dt.int16,
                             padded_shape=[128, rup(n_lo_blocks * 8, 256)])
            nc.sync.dma_start(lo_idx[:], p_loidx[:])
            hi_idx = pp.tile([128, n_hi_blocks * 8], mybir.dt.int16,
                             padded_shape=[128, rup(n_hi_blocks * 8, 256)])
            nc.sync.dma_start(hi_idx[:], p_hiidx[:])

            # matmul operands in f32r
            wxyzT = pp.tile([3, 128], f32)
            nc.sync.dma_start(wxyzT[:], p_wxyzT[:])
            wxyzT_r = pp.tile([3, 128], mdt)
            nc.vector.tensor_copy(wxyzT_r[:], wxyzT[:])
            wlatT_r = pp.tile([128, 4, 128], mdt)
            nc.vector.tensor_copy(wlatT_r[:], wlatT[:])
            latT_r = pp.tile([128, 4, B], mdt, padded_shape=[128, 4, 32])
            nc.vector.tensor_copy(latT_r[:], latT[:])
            convWT = pp.tile([128, 3, 128], f32)
            nc.sync.dma_start(convWT[:], p_convWT[:].rearrange("(l k) m -> k l m", l=3))
            convWT_r = pp.tile([128, 3, 128], mdt)
            nc.vector.tensor_copy(convWT_r[:], convWT[:])
            woutT = pp.tile([128, 3], f32, padded_shape=[128, 128])
            nc.sync.dma_start(woutT[:], p_woutT[:])
            woutT_r = pp.tile([128, 3], mdt, padded_shape=[128, 128])
            nc.vector.tensor_copy(woutT_r[:], woutT[:])
            eye = pp.tile([128, 128], f32)
            nc.sync.dma_start(eye[:], p_eye[:])
            eye_bf = pp.tile([128, 128], bf16, padded_shape=[128, 256])
            nc.vector.tensor_copy(eye_bf[:], eye[:])

            # c_all[:, b] = W_lat @ latent_b + b_in
            psc = ps_c.tile([128, B], f32)
            for ck in range(4):
                nc.tensor.matmul(psc[:], wlatT_r[:, ck, :], latT_r[:, ck, :],
                                 start=(ck == 0), stop=(ck == 3))
            c_all = pp.tile([128, B], f32, padded_shape=[128, 128])
            nc.vector.tensor_scalar(c_all[:], psc[:], b_in[:], None, op0=OP.add)

            # big feature-major activations (batch 0)
            X_fm = pp.tile([128, PER_CORE], mdt)     # current x
            G_fm = pp.tile([128, PER_CORE], bf16,
                           padded_shape=[128, PER_CORE + 128])  # dis*h (table src)
            tblnm = pp.tile([128, NT, 128], bf16)    # XBAR staging (node-major)

            g_own = dr.tile([PER_CORE, 128], bf16)
            tables = [
                dr.tile([TBL_ROWS, 128], bf16, addr_space="Shared", name=f"tbl{l}")
                for l in range(3)
            ]

            chunks = [(j, min(j + NCHUNK, PER_CORE)) for j in range(0, PER_CORE, NCHUNK)]

            # ---- batch-0 input layer: X_fm = relu(Wxyz@xyzT + c0) ------
            for j0, j1 in chunks:
                psx = ps_mm.tile([128, NCHUNK], f32, name="psx", tag="mm")
                n = j1 - j0
                xc = dn.tile([3, NCHUNK], mdt, name="xc", tag="xyzc")
                nc.sync.dma_start(xc[:, :n], p_xyzT[:, j0:j1])
                nc.tensor.matmul(psx[:, :n], wxyzT_r[:], xc[:, :n],
                                 start=True, stop=True)
                nc.scalar.activation(X_fm[:, j0:j1], psx[:, :n], AF.Relu,
                                     bias=c_all[:, 0:1])

            # ---- dense per-node MLP for one batch (fills collective gaps)
            def dense_batch(b):
                for j0, j1 in chunks:
                    n = j1 - j0
                    ps0 = ps_mm.tile([128, NCHUNK], f32, name="ps0", tag="mm")
                    xc = dn.tile([3, NCHUNK], mdt, name="xc", tag="xyzc")
                    nc.sync.dma_start(xc[:, :n], p_xyzT[:, j0:j1])
                    nc.tensor.matmul(ps0[:, :n], wxyzT_r[:], xc[:, :n],
                                     start=True, stop=True)
                    xb = dn.tile([128, NCHUNK], mdt, name="xb")
                    nc.scalar.activation(xb[:, :n], ps0[:, :n], AF.Relu,
                                         bias=c_all[:, b:b + 1])
                    for l in range(3):
                        psl = ps_mm.tile([128, NCHUNK], f32, name="ps0", tag="mm")
                        nc.tensor.matmul(psl[:, :n], convWT_r[:, l, :], xb[:, :n],
                                         start=True, stop=True)
                        xb = dn.tile([128, NCHUNK], mdt, name="xb")
                        nc.scalar.activation(xb[:, :n], psl[:, :n], AF.Relu,
                                             bias=convbT[:, l:l + 1])
                    pso = ps_mm.tile([3, NCHUNK], f32, name="pso", tag="mm")
                    nc.tensor.matmul(pso[:, :n], woutT_r[:], xb[:, :n],
                                     start=True, stop=True)
                    osb = dn.tile([3, NCHUNK], f32, name="osb")
                    nc.scalar.activation(osb[:, :n], pso[:, :n], AF.Identity,
                                         bias=b_out[:])
                    nc.sync.dma_start(p_out[b, :, j0:j1], osb[:, :n])

            if DEBUG:
                nc.sync.dma_start(p_dbg_x0[:], X_fm[:])

            # per-tile table build: H(t) = W_l @ X(t); G = dis*H;
            # PE-transpose into the node-major tblnm staging slice.
            def build_tile(l, t):
                ts0, ts1 = t * 128, t * 128 + 128
                psh = ps_mm.tile([128, NCHUNK], f32, name="psh", tag="mm")
                nc.tensor.matmul(psh[:, :128], convWT_r[:, l, :], X_fm[:, ts0:ts1],
                                 start=True, stop=True)
                nc.vector.tensor_tensor(G_fm[:, ts0:ts1], psh[:, :128],
                                        disb[:, ts0:ts1], op=OP.mult)
                pst = ps_t.tile([128, 128], bf16, name="pst", tag="pst")
                nc.tensor.transpose(pst[:], G_fm[:, ts0:ts1], eye_bf[:])
                nc.scalar.activation(tblnm[:, t, :], pst[:], AF.Copy)

            # ---- conv layers for batch 0 -------------------------------
            for t in range(NT):
                build_tile(0, t)
            for l in range(3):
                gw = nc.sync.dma_start(
                    g_own[:].rearrange("(j t) f -> j (t f)", j=128),
                    tblnm[:].rearrange("p t f -> p (t f)"))
                nc.gpsimd.collective_compute(
                    "AllGather", OP.bypass,
                    replica_groups=[list(range(NCORES))],
                    ins=[g_own[:]], outs=[tables[l][:]])

                # dense batch l+1 overlaps the collective + gathers
                dense_batch(l + 1)
                if DEBUG and l == 0:
                    nc.sync.dma_start(p_dbg_g[:], G_fm[:])
                    nc.sync.dma_start(p_dbg_tbl[:], tables[0][:])
                    dtn = nc.sync.dma_start(p_dbg_tn[:], tblnm[:].rearrange("p t f -> p (t f)"))
                    nc.sync.dma_start(p_dbg_go[:], g_own[:])

                tbl_lo = tables[l][0:32768, :]
                tbl_hi = tables[l][W2_BASE:W2_BASE + 32768, :]

                with nc.allow_low_precision("bf16 segment sums, tolerance 2e-2"):
                    for gi, (t0, t1) in enumerate(groups):
                        glo = int(Klo[t0:t1].sum())
                        ghi = int(Khi[t0:t1].sum())
                        hb = glo + (glo % 2)          # hi region start (aligned)
                        nb = hb + ghi
                        qn = (2 * gi) % NQUEUES
                        qn2 = (2 * gi + 1) % NQUEUES
                        slots = sl.tile([128, GROUP_BLOCKS, 128], bf16,
                                        name="slots", tag="slots")
                        if glo:
                            nidx = 128 * glo
                            c0 = int(lo_off[t0]) * 8
                            nc.gpsimd.dma_gather(
                                slots[:, 0:glo, :], tbl_lo,
                                lo_idx[:, c0:c0 + glo * 8], nidx, nidx, 128,
                                elem_step=128, single_packet=False, queue_num=qn)
                        if ghi:
                            nidx = 128 * ghi
                            c0 = int(hi_off[t0]) * 8
                            nc.gpsimd.dma_gather(
                                slots[:, hb:nb, :], tbl_hi,
                                hi_idx[:, c0:c0 + ghi * 8], nidx, nidx, 128,
                                elem_step=128, single_packet=False, queue_num=qn2)
                        if DEBUG and l == 0 and gi == 0:
                            nc.sync.dma_start(
                                p_dbg_s[:],
                                slots[:].rearrange("p k f -> p (k f)"))
                        for t in range(t0, t1):
                            la = int(Klo[t0:t].sum())
                            ha = hb + int(Khi[t0:t].sum())
                            kl, kh = int(Klo[t]), int(Khi[t])
                            ts0, ts1 = t * 128, t * 128 + 128
                            parts = []
                            if kl:
                                r1 = wk.tile([128, 128], rdt, name="r1",
                                             padded_shape=[128, 256])
                                nc.vector.tensor_reduce(
                                    r1[:],
                                    slots[:, la:la + kl, :]
                                    .rearrange("p k f -> p f k"),
                                    axis=AX.X, op=OP.add)
                                parts.append(r1)
                            if kh:
                                r2 = wk.tile([128, 128], rdt, name="r2",
                                             padded_shape=[128, 256])
                                nc.vector.tensor_reduce(
                                    r2[:],
                                    slots[:, ha:ha + kh, :]
                                    .rearrange("p k f -> p f k"),
                                    axis=AX.X, op=OP.add)
                                parts.append(r2)
                            # self term: dis * G (node-major from tblnm)
                            t2_ = wk.tile([128, 128], f32, name="t2")
                            nc.scalar.activation(t2_[:], tblnm[:, t, :], AF.Copy,
                                                 scale=dis_pp[:, t:t + 1])
                            if len(parts) == 2:
                                s = wk.tile([128, 128], f32, name="s")
                                nc.vector.tensor_tensor(
                                    s[:], parts[0][:], parts[1][:], op=OP.add)
                            elif parts:
                                s = parts[0]
                            else:
                                s = None
                            u = wk.tile([128, 128], f32, name="u")
                            if s is not None:
                                t1_ = wk.tile([128, 128], f32, name="t1")
                                nc.scalar.activation(t1_[:], s[:], AF.Copy,
                                                     scale=disB_pp[:, t:t + 1])
                                nc.vector.tensor_tensor(
                                    u[:], t1_[:], t2_[:], op=OP.add)
                            else:
                                nc.vector.tensor_copy(u[:], t2_[:])
                            # back to feature-major: x' = relu(u^T + b_l)
                            pstu = ps_t.tile([128, 128], f32, name="pstu",
                                             tag="pstu")
                            nc.tensor.transpose(pstu[:], u[:], eye[:])
                            nc.scalar.activation(X_fm[:, ts0:ts1], pstu[:],
                                                 AF.Relu,
                                                 bias=convbT[:, l:l + 1])
                            if l < 2:
                                build_tile(l + 1, t)

                if DEBUG and l == 0:
                    nc.sync.dma_start(p_dbg_x1[:], X_fm[:])

            # ---- batch-0 output layer ----------------------------------
            for j0, j1 in chunks:
                n = j1 - j0
                pso = ps_mm.tile([3, NCHUNK], f32, name="pso2", tag="mm")
                nc.tensor.matmul(pso[:, :n], woutT_r[:], X_fm[:, j0:j1],
                                 start=True, stop=True)
                osb = dn.tile([3, NCHUNK], f32, name="osb2")
                nc.scalar.activation(osb[:, :n], pso[:, :n], AF.Identity,
                                     bias=b_out[:])
                nc.sync.dma_start(p_out[0, :, j0:j1], osb[:, :n])

    nc.compile()
    return nc


def kernel(**inputs):
    global LAST_EXEC_NS
    from concourse.bass_utils import run_bass_kernel_spmd

    edge_index = np.asarray(inputs["edge_index"])
    key = hash(edge_index.tobytes())
    if key not in _CACHE:
        meta = _preprocess(edge_index)
        nc = _build(meta)
        _CACHE[key] = (meta, nc)
    meta, nc = _CACHE[key]

    xyz = np.asarray(inputs["vertex_xyz"], np.float32)
    lat = np.asarray(inputs["latent"], np.float32)
    W_in = np.asarray(inputs["W_in"], np.float32)
    b_in = np.asarray(inputs["b_in"], np.float32)
    conv_W = np.asarray(inputs["conv_W"], np.float32)
    conv_b = np.asarray(inputs["conv_b"], np.float32)
    W_out = np.asarray(inputs["W_out"], np.float32)
    b_out = np.asarray(inputs["b_out"], np.float32)

    nodes = meta["nodes"]
    shared = dict(
        eye=np.eye(128, dtype=np.float32),
        wxyzT=np.ascontiguousarray(W_in[:, :3].T),            # (3,128)
        wlatT=np.ascontiguousarray(W_in[:, 3:].T),            # (512,128)
        latT=np.ascontiguousarray(lat.T),                     # (512,4)
        b_in=b_in.reshape(128, 1),
        convWT=np.ascontiguousarray(
            np.concatenate([conv_W[l].T for l in range(3)], axis=0)),
        convbT=np.ascontiguousarray(conv_b.T),
        woutT=np.ascontiguousarray(W_out.T),
        b_out=b_out.reshape(3, 1),
    )

    in_maps = []
    for c in range(NCORES):
        nc_nodes = nodes[c]
        valid = nc_nodes >= 0
        xyz_c = np.zeros((PER_CORE, 3), np.float32)
        xyz_c[valid] = xyz[nc_nodes[valid]]
        m = dict(shared)
        xt = np.ascontiguousarray(xyz_c.T)
        xb = xt.view(np.uint32)
        xb[:] = (xb + 0x1000) & np.uint32(0xFFFFE000)   # round to fp32r
        m["xyzT"] = xt
        dis = meta["dis_row"][c]                         # (PER_CORE,)
        m["disb"] = np.ascontiguousarray(
            np.broadcast_to(dis[None, :], (128, PER_CORE)))
        dpp = np.ascontiguousarray(dis.reshape(NT, 128).T)       # (128, NT)
        m["dis_pp"] = dpp
        m["disB_pp"] = np.ascontiguousarray(dpp * float(B))
        m["lo_idx"] = meta["lo_wrapped"][c]
        m["hi_idx"] = meta["hi_wrapped"][c]
        in_maps.append(m)

    trace = os.environ.get("GCN_TRACE", "0") == "1"
    if trace:
        _install_ntff_hook()
    res = run_bass_kernel_spmd(nc, in_maps, list(range(NCORES)), trace=trace)
    LAST_EXEC_NS = res.exec_time_ns

    out = np.empty((B, V, 3), np.float32)
    for c in range(NCORES):
        rn = nodes[c, :REAL_PER_CORE]
        oc = res.results[c]["out_all"]          # (B, 3, PER_CORE)
        out[:, rn, :] = oc[:, :, :REAL_PER_CORE].transpose(0, 2, 1)
    return out
